# revision 2
# baseline (speedup 1.0000x reference)
"""Trainium2 Bass kernel for nn_AttentionBlock (B=32, C=256, H=W=32).

Data-parallel over batch across 8 NeuronCores (4 batch elements per core);
all parameters replicated.

Algorithm per batch element (x: [C=256, N=1024]):
  h  = GroupNorm(x; 8 groups) * gn_w + gn_b
  q  = (wq/sqrt(C)) @ h + bq/sqrt(C)          [C, N]   (scale folded into wq)
  k  = wk @ h + bk                            [C, N]
  vT = hT @ wvT                               [N, C]   (bv folded into bp!)
  ST[j,i] = sum_c k[c,j] q[c,i]               [N, N]   (scores, transposed)
  E  = exp(ST)            (scores are in [-9, 9] for this model; no max-sub)
  rowsum[i] = sum_j E[j,i]  via bf16 pair-tree adds + one ones-matmul
  outU[c,i] = sum_j vT[j,c] E[j,i]            (PSUM accum over j-tiles)
  y  = x + wp @ (outU * (1/rowsum)) + (bp + wp@bv)

The transposed-score formulation needs no [N,N] transposes.  The rowsum
is built by summing the 8 exp j-tiles with two bf16 add-chains on an
auxiliary engine (GpSimd by default; it is otherwise idle) and a single
ones-stationary matmul that both reduces over partitions and replicates
the result across all 128 partitions, so the softmax reciprocal runs
wide on the VectorEngine with no partition broadcast.  This removes 7/8
of the rowsum TensorEngine traffic.  bv is applied as wp@bv added to bp
on the host (softmax weights sum to 1), removing the bias matmuls in v.

GroupNorm heads are software-pipelined two batches ahead of their qkv
so the DVE bn-statistics chains for batch b+2 hide under the attention
matmul stream of batch b, and the TensorEngine never idles long enough
for the HAM clock gate to re-throttle (PE idle >3.4us -> 1.2GHz).
A dummy matmul burst on a memset tile warms the PE activity monitor
during the initial DMA/GroupNorm ramp.  The last batch's final i-half
is emitted in column chunks so the closing normalize/proj/residual/DMA
chain pipelines instead of serializing.
"""

import numpy as np

import concourse.bacc as bacc
import concourse.bass as bass
import concourse.mybir as mybir
import concourse.tile as tile
from concourse.bass_utils import run_bass_kernel_spmd

B, C, HH, WW = 32, 256, 32, 32
N = HH * WW                 # 1024 spatial positions
NCORES = 8
BPC = B // NCORES           # batch elements per core
G = 8                       # groupnorm groups
GS = C // G                 # channels per group
P = 128                     # SBUF partitions
NCH = C // P                # channel chunks (2)
IH = 512                    # i-half width (PSUM bank is 512 fp32)
NIH = N // IH               # 2
NJ = N // P                 # 8 j-tiles
EPS = 1e-5

F32 = mybir.dt.float32
BF16 = mybir.dt.bfloat16
SIG_DT = BF16               # groupnorm h, q/k + weights (score precision)
VAL_DT = BF16               # exp(S), vT, normalized out, wp weights
# rowsum strategy: 'gpsimd' / 'vector' = pair-tree adds on that engine +
# 1 ones-matmul per i-half; 'pe' = 8 ones-matmuls per i-half (baseline)
ESUM = 'gpsimd'
TAIL_CHUNKS = 2             # column chunks for the last batch's last i-half
AF = mybir.ActivationFunctionType
OP = mybir.AluOpType


def build_kernel_body(nc, tc, x_d, y_d, wd, spack_d, indT_d, ones_d):
    ctxpools = dict(
        const=tc.tile_pool(name="const", bufs=1),
        xp=tc.tile_pool(name="xp", bufs=1),
        hp=tc.tile_pool(name="hp", bufs=4),
        qk=tc.tile_pool(name="qk", bufs=3),
        vtp=tc.tile_pool(name="vtp", bufs=3),
        etp=tc.tile_pool(name="etp", bufs=2),
        esp=tc.tile_pool(name="esp", bufs=2),
        sm=tc.tile_pool(name="sm", bufs=4),
        outp=tc.tile_pool(name="outp", bufs=2),
        pp=tc.tile_pool(name="pp", bufs=8, space=bass.MemorySpace.PSUM),
    )
    pools = {k: v.__enter__() for k, v in ctxpools.items()}
    const = pools["const"]
    pp = pools["pp"]
    sm = pools["sm"]
    es_eng = nc.gpsimd if ESUM == 'gpsimd' else nc.vector

    # ---- input + constant loads, spread across issue queues ----
    # DMA descriptor issue costs ~0.6us each and serializes per engine; x
    # goes first on Sync (unblocks GroupNorm), weights on Scalar, packed
    # small constants on GpSimd.
    st = {}   # per-batch tiles: xt, ht, qt, kt, vt, fin
    for b in range(BPC):
        xt = []
        for ch in range(NCH):
            t = pools["xp"].tile([P, N], F32, name=f"xt{b}_{ch}", tag=f"xt{b}_{ch}")
            if b == 0:
                # halves: GroupNorm's first bn_stats starts ~1.5us earlier
                for hh in range(2):
                    nc.sync.dma_start(out=t[:, hh * IH:(hh + 1) * IH],
                                      in_=x_d[b, ch * P:(ch + 1) * P, hh * IH:(hh + 1) * IH])
            else:
                nc.sync.dma_start(out=t, in_=x_d[b, ch * P:(ch + 1) * P, :])
            xt.append(t)
        st[b] = dict(xt=xt)

    wt = {}   # weights, transposed: [c_chunk][128, 256]
    for name in ("q", "k", "v", "p"):
        wt[name] = []
        for ch in range(NCH):
            wdt = VAL_DT if name == "p" else SIG_DT
            w_tile = const.tile([P, C], wdt, tag=f"w{name}{ch}")
            nc.scalar.dma_start(out=w_tile, in_=wd[name][ch * P:(ch + 1) * P, :])
            wt[name].append(w_tile)
    ones128 = const.tile([P, P], VAL_DT, tag="ones128")
    nc.scalar.dma_start(out=ones128, in_=ones_d[:, :])

    # one packed DMA for all per-partition scalars + group indicators:
    # cols 0-5 = bq0,bq1,bk0,bk1,bp0,bp1; 6-7 gnw; 8-9 gnb; 10-25 ind chunks
    spack = const.tile([P, 26], F32, tag="spack")
    nc.gpsimd.dma_start(out=spack, in_=spack_d[:, :])
    bt = {"q": [spack[:, 0:1], spack[:, 1:2]],
          "k": [spack[:, 2:3], spack[:, 3:4]],
          "p": [spack[:, 4:5], spack[:, 5:6]]}
    gnw_t = [spack[:, 6:7], spack[:, 7:8]]
    gnb_t = [spack[:, 8:9], spack[:, 9:10]]
    ind_t = [spack[:, 10:18], spack[:, 18:26]]

    indT_t = []
    for ch in range(NCH):
        itT = const.tile([G, P], F32, tag=f"indT{ch}")
        nc.gpsimd.dma_start(out=itT, in_=indT_d[:, ch * P:(ch + 1) * P])
        indT_t.append(itT)
    eps8 = const.tile([G, 1], F32, tag="eps8")
    nc.vector.memset(eps8, EPS)
    sqrt_warm = const.tile([G, 1], F32, tag="sqrt_warm")
    nc.scalar.activation(out=sqrt_warm, in_=eps8, func=AF.Sqrt, bias=eps8, scale=1.0)

    # HAM warm-up: back-to-back matmuls on a memset tile keep the PE busy
    # during the DMA/GroupNorm ramp so the activity monitor unthrottles
    # the clock (1.2 -> 2.4 GHz) before real matmuls arrive
    warm_in = const.tile([P, IH], VAL_DT, tag="warm_in")
    nc.vector.memset(warm_in, 1.0)
    wpsum = pp.tile([P, IH], F32, tag="ps")
    for _ in range(24):
        nc.tensor.matmul(wpsum, warm_in[:, 0:P], warm_in, start=True, stop=True)
    warm_sink = const.tile([P, 1], F32, tag="warm_sink")
    nc.vector.tensor_copy(out=warm_sink, in_=wpsum[:, 0:1])

    # ---- per-batch pipeline, software-pipelined across batches ----

    def emit_head(b):
        xt = st[b]["xt"]

        # -- GroupNorm statistics --
        # per-channel mean / E[x^2] over the 1024 free elements
        pcs = []
        for ch in range(NCH):
            stats = sm.tile([P, 2, 6], F32, tag="bnstats")
            for sg in range(2):
                nc.vector.bn_stats(out=stats[:, sg, :], in_=xt[ch][:, sg * 512:(sg + 1) * 512])
            mv = sm.tile([P, 2], F32, tag="mv")
            nc.vector.bn_aggr(out=mv, in_=stats)
            pc = sm.tile([P, 2], F32, tag=f"pc{ch}")
            nc.vector.tensor_copy(out=pc[:, 0:1], in_=mv[:, 0:1])
            nc.vector.scalar_tensor_tensor(out=pc[:, 1:2], in0=mv[:, 0:1],
                                           scalar=mv[:, 0:1], in1=mv[:, 1:2],
                                           op0=OP.mult, op1=OP.add)  # mean^2 + var
            pcs.append(pc)
        # group-reduce across the 32 channels of each group (partition axis)
        pg = pp.tile([G, 2], F32, tag="ps")
        for ch in range(NCH):
            nc.tensor.matmul(pg, ind_t[ch], pcs[ch], start=(ch == 0), stop=(ch == NCH - 1))
        br8 = sm.tile([G, 2], F32, tag="br8")   # [:,0]=mean_g  [:,1]=rstd_g
        nc.vector.tensor_scalar_mul(out=br8, in0=pg, scalar1=1.0 / 32.0)
        m2g = sm.tile([G, 1], F32, tag="m2g")
        nc.vector.tensor_mul(m2g, br8[:, 0:1], br8[:, 0:1])
        nc.vector.tensor_sub(br8[:, 1:2], br8[:, 1:2], m2g)    # var_g
        nc.scalar.activation(out=br8[:, 1:2], in_=br8[:, 1:2], func=AF.Sqrt, bias=eps8, scale=1.0)
        nc.vector.reciprocal(out=br8[:, 1:2], in_=br8[:, 1:2])

        # broadcast group stats back to channels, fold gn affine, normalize
        ht = []
        for ch in range(NCH):
            pbc = pp.tile([P, 2], F32, tag="ps")
            nc.tensor.matmul(pbc, indT_t[ch], br8)
            s_ = sm.tile([P, 1], F32, tag=f"s{ch}")
            t_ = sm.tile([P, 1], F32, tag=f"t{ch}")
            nc.vector.tensor_mul(s_, pbc[:, 1:2], gnw_t[ch])   # s = rstd * w
            nc.vector.scalar_tensor_tensor(out=t_, in0=pbc[:, 0:1], scalar=s_,
                                           in1=gnb_t[ch], op0=OP.mult,
                                           op1=OP.subtract)    # t = mean*s - b
            h_ = pools["hp"].tile([P, N], SIG_DT, name=f"ht{ch}", tag=f"ht{ch}")
            nc.vector.tensor_scalar(
                out=h_, in0=xt[ch], scalar1=s_, scalar2=t_,
                op0=OP.mult, op1=OP.subtract)  # x*s - t
            ht.append(h_)
        st[b]["ht"] = ht

    def emit_qkv(b):
        ht = st[b]["ht"]
        # -- q, k projections: [C, N] = W^T.T @ h (+ bias during PSUM move) --
        # i-half-major so attention on i-half 0 starts after only 4 moves
        qt = [pools["qk"].tile([P, N], SIG_DT, name=f"qt{och}", tag=f"qt{och}")
              for och in range(NCH)]
        kt = [pools["qk"].tile([P, N], SIG_DT, name=f"kt{och}", tag=f"kt{och}")
              for och in range(NCH)]
        for ih in range(NIH):
            for wname, dst in (("q", qt), ("k", kt)):
                for och in range(NCH):
                    pq = pp.tile([P, IH], F32, tag="ps")
                    for cch in range(NCH):
                        nc.tensor.matmul(
                            pq,
                            wt[wname][cch][:, och * P:(och + 1) * P],
                            ht[cch][:, ih * IH:(ih + 1) * IH],
                            start=(cch == 0), stop=(cch == NCH - 1))
                    if wname == "k":
                        nc.scalar.add(out=dst[och][:, ih * IH:(ih + 1) * IH],
                                      in_=pq, add=bt[wname][och])
                    else:
                        nc.vector.tensor_scalar_add(
                            out=dst[och][:, ih * IH:(ih + 1) * IH], in0=pq,
                            scalar1=bt[wname][och])

        # -- v, produced transposed: vT[n, o] = h[:, n].T @ wvT  (bv folded
        # into bp on the host: softmax weights sum to 1, so the +bv term
        # passes through attention unchanged and commutes with wp) --
        vt = []
        for j in range(NJ):
            pv = pp.tile([P, C], F32, tag="ps")
            for cch in range(NCH):
                nc.tensor.matmul(pv, ht[cch][:, j * P:(j + 1) * P], wt["v"][cch],
                                 start=(cch == 0), stop=(cch == NCH - 1))
            v_ = pools["vtp"].tile([P, C], VAL_DT, name=f"vt{j}", tag=f"vt{j}")
            nc.scalar.copy(out=v_, in_=pv)
            vt.append(v_)
        st[b].update(qt=qt, kt=kt, vt=vt)

    def emit_attn_scores(b, ih):
        qt, kt, vt = (st[b][k] for k in ("qt", "kt", "vt"))
        if ih == 0:
            st[b]["fin"] = [pools["outp"].tile([P, N], F32, name=f"fin{och}",
                                               tag=f"fin{och}") for och in range(NCH)]
        isl = slice(ih * IH, (ih + 1) * IH)
        po = [pp.tile([P, IH], F32, name=f"po{_}", tag="ps") for _ in range(NCH)]
        ets = [None] * NJ
        # rowsum accumulators: two bf16 add-chains (even/odd j) + combine
        chain = {0: [], 1: []}

        def s_stage(j):
            ps = pp.tile([P, IH], F32, tag="ps")
            for cch in range(NCH):
                nc.tensor.matmul(ps,
                                 kt[cch][:, j * P:(j + 1) * P],
                                 qt[cch][:, isl],
                                 start=(cch == 0), stop=(cch == NCH - 1))
            et = pools["etp"].tile([P, IH], VAL_DT, name=f"et{j}", tag=f"et{j}")
            nc.scalar.activation(out=et, in_=ps, func=AF.Exp)
            ets[j] = et

        def es_add(j):
            # after et[j] lands, extend that parity's chain by one add
            par = j & 1
            if j < 2:
                chain[par].append(ets[j])
                return
            acc = pools["esp"].tile([P, IH], VAL_DT, tag=f"es{par}_{j // 2}")
            es_eng.tensor_add(acc, chain[par][-1], ets[j])
            chain[par].append(acc)

        def acc_stage(j):
            et = ets[j]
            if ESUM == 'pe':
                nc.tensor.matmul(prs, ones128, et, start=(j == 0), stop=(j == NJ - 1))
            for och in range(NCH):
                nc.tensor.matmul(po[och], vt[j][:, och * P:(och + 1) * P], et,
                                 start=(j == 0), stop=(j == NJ - 1))

        if ESUM == 'pe':
            prs = pp.tile([P, IH], F32, name="prs", tag="ps")
            s_stage(0)
            for j in range(1, NJ):
                s_stage(j)
                acc_stage(j - 1)
            acc_stage(NJ - 1)
        else:
            s_stage(0)
            for j in range(1, NJ):
                s_stage(j)
                es_add(j - 1)
                acc_stage(j - 1)
            es_add(NJ - 1)
            acc_stage(NJ - 1)
            esum = pools["esp"].tile([P, IH], VAL_DT, tag="esum")
            es_eng.tensor_add(esum, chain[0][-1], chain[1][-1])
            prs = pp.tile([P, IH], F32, name="prs", tag="ps")
            nc.tensor.matmul(prs, ones128, esum, start=True, stop=True)
        st[b][f"acc{ih}"] = (prs, po)

    def emit_attn_norm(b, ih, csl=None, cn=''):
        prs, po = st[b][f"acc{ih}"]
        csl = csl if csl is not None else slice(0, IH)
        w = csl.stop - csl.start
        rb = sm.tile([P, w], F32, tag="rb" + cn)
        rscratch = sm.tile([P, w], F32, tag="rscratch" + cn)
        nc.vector.reciprocal_approx_accurate(out=rb, in_=prs[:, csl], scratch=rscratch)
        ou = []
        for cch in range(NCH):
            o_ = pools["outp"].tile([P, w], VAL_DT, name=f"ou{cch}", tag=f"ou{cch}{cn}")
            nc.vector.tensor_mul(o_, po[cch][:, csl], rb)           # normalize
            ou.append(o_)
        st[b][f"ou{ih}{cn}"] = ou

    def emit_attn_out(b, ih, csl=None, cn=''):
        xt, fin = st[b]["xt"], st[b]["fin"]
        ou = st[b][f"ou{ih}{cn}"]
        csl = csl if csl is not None else slice(0, IH)
        isl = slice(ih * IH + csl.start, ih * IH + csl.stop)
        for och in range(NCH):
            pz = pp.tile([P, csl.stop - csl.start], F32, tag="ps")
            for cch in range(NCH):
                nc.tensor.matmul(pz,
                                 wt["p"][cch][:, och * P:(och + 1) * P],
                                 ou[cch],
                                 start=(cch == 0), stop=(cch == NCH - 1))
            # y = (wp@ou + bp') + x   in one fused DVE pass
            nc.vector.scalar_tensor_tensor(
                out=fin[och][:, isl], in0=pz, scalar=bt["p"][och],
                in1=xt[och][:, isl], op0=OP.add, op1=OP.add)
            nc.sync.dma_start(out=y_d[b, och * P:(och + 1) * P, isl],
                              in_=fin[och][:, isl])

    # heads pipelined two batches ahead of their qkv: head(b+2) hides
    # under the attention matmul stream of batch b, so neither the PE
    # (waiting on evacuations stuck behind bn chains) nor the HAM clock
    # gate (PE idle >3.4us) suffers.
    emit_head(0)
    emit_qkv(0)
    emit_head(1)
    for b in range(BPC):
        emit_attn_scores(b, 0)
        emit_attn_norm(b, 0)
        emit_attn_scores(b, 1)
        if b + 2 < BPC:
            emit_head(b + 2)
        emit_attn_out(b, 0)
        if b + 1 < BPC:
            emit_attn_norm(b, 1)
            emit_qkv(b + 1)
            emit_attn_out(b, 1)
        else:
            # closing chain: chunk columns so norm/proj/residual/DMA pipeline
            cw = IH // TAIL_CHUNKS
            for c in range(TAIL_CHUNKS):
                csl = slice(c * cw, (c + 1) * cw)
                emit_attn_norm(b, 1, csl, cn=f"c{c}")
                emit_attn_out(b, 1, csl, cn=f"c{c}")
        del st[b]

    for k in reversed(list(ctxpools)):
        ctxpools[k].__exit__(None, None, None)


def build_bass():
    nc = bacc.Bacc("TRN2", target_bir_lowering=False, debug=False)
    x_d = nc.dram_tensor("x", [BPC, C, N], F32, kind="ExternalInput")
    wd = {name: nc.dram_tensor(f"w{name}T", [C, C], VAL_DT if name == "p" else SIG_DT,
                               kind="ExternalInput")
          for name in ("q", "k", "v", "p")}
    spack_d = nc.dram_tensor("spack", [P, 26], F32, kind="ExternalInput")
    indT_d = nc.dram_tensor("indT", [G, C], F32, kind="ExternalInput")
    ones_d = nc.dram_tensor("ones", [P, P], VAL_DT, kind="ExternalInput")
    y_d = nc.dram_tensor("y", [BPC, C, N], F32, kind="ExternalOutput")

    with tile.TileContext(nc) as tc:
        build_kernel_body(nc, tc, x_d, y_d, wd, spack_d, indT_d, ones_d)
    nc.compile()
    return nc


def host_inputs(inputs):
    """Per-core replicated constants from the full input dict."""
    import ml_dtypes
    np_sig = np.float32 if SIG_DT != BF16 else ml_dtypes.bfloat16
    np_val = np.float32 if VAL_DT != BF16 else ml_dtypes.bfloat16
    f = lambda a: np.ascontiguousarray(np.asarray(a), dtype=np.float32)
    scale = np.float32(C ** -0.5)
    ind = np.zeros((C, G), dtype=np.float32)
    for c in range(C):
        ind[c, c // GS] = 1.0
    bq = f(inputs["bq"]) * scale
    bk = f(inputs["bk"])
    # bv commutes through the softmax (weights sum to 1): fold wp@bv into bp
    bp = f(inputs["bp"]) + f(inputs["wp"]) @ f(inputs["bv"])
    gnw = f(inputs["gn_w"])
    gnb = f(inputs["gn_b"])
    spack = np.zeros((P, 26), dtype=np.float32)
    for ch in range(NCH):
        sl = slice(ch * P, (ch + 1) * P)
        spack[:, 0 + ch] = bq[sl]
        spack[:, 2 + ch] = bk[sl]
        spack[:, 4 + ch] = bp[sl]
        spack[:, 6 + ch] = gnw[sl]
        spack[:, 8 + ch] = gnb[sl]
        spack[:, 10 + 8 * ch:18 + 8 * ch] = ind[sl, :]
    consts = {
        "wqT": f(np.asarray(inputs["wq"], dtype=np.float32).T * scale).astype(np_sig),
        "wkT": f(np.asarray(inputs["wk"], dtype=np.float32).T).astype(np_sig),
        "wvT": f(np.asarray(inputs["wv"], dtype=np.float32).T).astype(np_sig),
        "wpT": f(np.asarray(inputs["wp"], dtype=np.float32).T).astype(np_val),
        "spack": spack,
        "indT": np.ascontiguousarray(ind.T),
        "ones": np.ones((P, P), dtype=np_val),
    }
    return consts


_NC_CACHE = []


def _get_nc():
    if not _NC_CACHE:
        _NC_CACHE.append(build_bass())
    return _NC_CACHE[0]


def kernel(trace=False, trace_cores=None, **inputs):
    nc = _get_nc()
    consts = host_inputs(inputs)
    x = np.ascontiguousarray(np.asarray(inputs["x"], dtype=np.float32)).reshape(B, C, N)
    in_maps = []
    for core in range(NCORES):
        m = dict(consts)
        m["x"] = np.ascontiguousarray(x[core * BPC:(core + 1) * BPC])
        in_maps.append(m)
    res = run_bass_kernel_spmd(nc, in_maps, core_ids=list(range(NCORES)),
                               trace=trace, trace_cores=trace_cores)
    y = np.concatenate([r["y"] for r in res.results], axis=0)
    out = y.reshape(B, C, HH, WW).astype(np.float32)
    if trace:
        return out, res
    return out


# revision 7
# speedup vs baseline: 1.4858x; 1.4858x over previous
"""Trainium2 Bass kernel for nn_AttentionBlock (B=32, C=256, H=W=32).

Data-parallel over batch across 8 NeuronCores (4 batch elements per core);
all parameters replicated.

Algorithm per batch element (x: [C=256, N=1024]):
  h  = GroupNorm(x; 8 groups) * gn_w + gn_b
  q  = (wq/sqrt(C)) @ h + bq/sqrt(C)          [C, N]   (scale folded into wq)
  k  = wk @ h + bk                            [C, N]
  vT = hT @ wvT                               [N, C]   (bv folded into bp!)
  ST[j,i] = sum_c k[c,j] q[c,i]               [N, N]   (scores, transposed)
  E  = exp(ST)            (scores are in [-9, 9] for this model; no max-sub)
  rowsum[i] = sum_j E[j,i]  via bf16 pair-tree adds + one ones-matmul
  outU[c,i] = sum_j vT[j,c] E[j,i]            (PSUM accum over j-tiles)
  y  = x + wp @ (outU * (1/rowsum)) + (bp + wp@bv)

The transposed-score formulation needs no [N,N] transposes.  The rowsum
is built by summing the 8 exp j-tiles with two bf16 add-chains on an
auxiliary engine (GpSimd by default; it is otherwise idle) and a single
ones-stationary matmul that both reduces over partitions and replicates
the result across all 128 partitions, so the softmax reciprocal runs
wide on the VectorEngine with no partition broadcast.  This removes 7/8
of the rowsum TensorEngine traffic.  bv is applied as wp@bv added to bp
on the host (softmax weights sum to 1), removing the bias matmuls in v.

GroupNorm heads are software-pipelined two batches ahead of their qkv
so the DVE bn-statistics chains for batch b+2 hide under the attention
matmul stream of batch b, and the TensorEngine never idles long enough
for the HAM clock gate to re-throttle (PE idle >3.4us -> 1.2GHz).
A dummy matmul burst on a memset tile warms the PE activity monitor
during the initial DMA/GroupNorm ramp.  The last batch's final i-half
is emitted in column chunks so the closing normalize/proj/residual/DMA
chain pipelines instead of serializing.
"""

import numpy as np

import concourse.bacc as bacc
import concourse.bass as bass
import concourse.mybir as mybir
import concourse.tile as tile
from concourse.bass_utils import run_bass_kernel_spmd

B, C, HH, WW = 32, 256, 32, 32
N = HH * WW                 # 1024 spatial positions
NCORES = 8
BPC = B // NCORES           # batch elements per core
G = 8                       # groupnorm groups
GS = C // G                 # channels per group
P = 128                     # SBUF partitions
NCH = C // P                # channel chunks (2)
IH = 512                    # i-half width (PSUM bank is 512 fp32)
NIH = N // IH               # 2
NJ = N // P                 # 8 j-tiles
EPS = 1e-5

F32 = mybir.dt.float32
BF16 = mybir.dt.bfloat16
SIG_DT = BF16               # groupnorm h, q/k + weights (score precision)
VAL_DT = BF16               # exp(S), vT, normalized out, wp weights
# rowsum strategy: 'gpsimd' / 'vector' = pair-tree adds on that engine +
# 1 ones-matmul per i-half; 'pe' = 8 ones-matmuls per i-half (baseline).
# Measured: gpsimd tensor_tensor is ~1.4us per [128,512] tile AND its SBUF
# traffic slows DVE/ACT by ~20% across the board — keep the PE version.
ESUM = 'pe'
TAIL_CHUNKS = 2             # column chunks for the last batch's last i-half
AF = mybir.ActivationFunctionType
OP = mybir.AluOpType


def build_kernel_body(nc, tc, x_d, y_d, wd, spack_d, indT_d, ones_d):
    ctxpools = dict(
        const=tc.tile_pool(name="const", bufs=1),
        xp=tc.tile_pool(name="xp", bufs=1),
        hp=tc.tile_pool(name="hp", bufs=4),
        qk=tc.tile_pool(name="qk", bufs=3),
        vtp=tc.tile_pool(name="vtp", bufs=3),
        etp=tc.tile_pool(name="etp", bufs=2),
        esp=tc.tile_pool(name="esp", bufs=2),
        sm=tc.tile_pool(name="sm", bufs=4),
        outp=tc.tile_pool(name="outp", bufs=2),
        pp=tc.tile_pool(name="pp", bufs=8, space=bass.MemorySpace.PSUM),
    )
    pools = {k: v.__enter__() for k, v in ctxpools.items()}
    const = pools["const"]
    pp = pools["pp"]
    sm = pools["sm"]
    es_eng = nc.gpsimd if ESUM == 'gpsimd' else nc.vector

    # ---- input + constant loads, spread across issue queues ----
    # DMA descriptor issue costs ~0.6us each and serializes per engine; x
    # goes first on Sync (unblocks GroupNorm), weights on Scalar, packed
    # small constants on GpSimd.
    st = {}   # per-batch tiles: xt, ht, qt, kt, vt, fin
    for b in range(BPC):
        xt = []
        for ch in range(NCH):
            t = pools["xp"].tile([P, N], F32, name=f"xt{b}_{ch}", tag=f"xt{b}_{ch}")
            if b == 0:
                # halves: GroupNorm's first bn_stats starts ~1.5us earlier
                for hh in range(2):
                    nc.sync.dma_start(out=t[:, hh * IH:(hh + 1) * IH],
                                      in_=x_d[b, ch * P:(ch + 1) * P, hh * IH:(hh + 1) * IH])
            else:
                nc.sync.dma_start(out=t, in_=x_d[b, ch * P:(ch + 1) * P, :])
            xt.append(t)
        st[b] = dict(xt=xt)

    wt = {}   # weights, transposed: [c_chunk][128, 256]
    for name in ("q", "k", "v", "p"):
        wt[name] = []
        for ch in range(NCH):
            wdt = VAL_DT if name == "p" else SIG_DT
            w_tile = const.tile([P, C], wdt, tag=f"w{name}{ch}")
            nc.scalar.dma_start(out=w_tile, in_=wd[name][ch * P:(ch + 1) * P, :])
            wt[name].append(w_tile)
    ones128 = const.tile([P, P], VAL_DT, tag="ones128")
    nc.scalar.dma_start(out=ones128, in_=ones_d[:, :])

    # one packed DMA for all per-partition scalars + group indicators:
    # cols 0-5 = bq0,bq1,bk0,bk1,bp0,bp1; 6-7 gnw; 8-9 gnb; 10-25 ind chunks
    spack = const.tile([P, 26], F32, tag="spack")
    nc.gpsimd.dma_start(out=spack, in_=spack_d[:, :])
    bt = {"q": [spack[:, 0:1], spack[:, 1:2]],
          "k": [spack[:, 2:3], spack[:, 3:4]],
          "p": [spack[:, 4:5], spack[:, 5:6]]}
    gnw_t = [spack[:, 6:7], spack[:, 7:8]]
    gnb_t = [spack[:, 8:9], spack[:, 9:10]]
    ind_t = [spack[:, 10:18], spack[:, 18:26]]

    indT_t = []
    for ch in range(NCH):
        itT = const.tile([G, P], F32, tag=f"indT{ch}")
        nc.gpsimd.dma_start(out=itT, in_=indT_d[:, ch * P:(ch + 1) * P])
        indT_t.append(itT)
    eps8 = const.tile([G, 1], F32, tag="eps8")
    nc.vector.memset(eps8, EPS)
    sqrt_warm = const.tile([G, 1], F32, tag="sqrt_warm")
    nc.scalar.activation(out=sqrt_warm, in_=eps8, func=AF.Sqrt, bias=eps8, scale=1.0)

    # HAM warm-up: back-to-back matmuls on a memset tile keep the PE busy
    # during the DMA/GroupNorm ramp so the activity monitor unthrottles
    # the clock (1.2 -> 2.4 GHz) before real matmuls arrive
    warm_in = const.tile([P, IH], VAL_DT, tag="warm_in")
    nc.vector.memset(warm_in, 1.0)
    wpsum = pp.tile([P, IH], F32, tag="ps")
    for _ in range(24):
        nc.tensor.matmul(wpsum, warm_in[:, 0:P], warm_in, start=True, stop=True)
    warm_sink = const.tile([P, 1], F32, tag="warm_sink")
    nc.vector.tensor_copy(out=warm_sink, in_=wpsum[:, 0:1])

    # ---- per-batch pipeline, software-pipelined across batches ----

    def emit_head_stats(b):
        # GroupNorm statistics: per-channel mean / E[x^2], group-reduce on
        # the partition axis via indicator matmuls, then sqrt+reciprocal.
        # The Sqrt runs on ScalarE: ALL batches' stats are emitted before
        # the first attention Exp so the ACT function table never thrashes
        # mid-kernel (a table switch is ~1.5us).
        xt = st[b]["xt"]
        pcs = []
        for ch in range(NCH):
            stats = sm.tile([P, 2, 6], F32, tag="bnstats")
            for sg in range(2):
                nc.vector.bn_stats(out=stats[:, sg, :], in_=xt[ch][:, sg * 512:(sg + 1) * 512])
            mv = sm.tile([P, 2], F32, tag="mv")
            nc.vector.bn_aggr(out=mv, in_=stats)
            pc = sm.tile([P, 2], F32, tag=f"pc{ch}")
            nc.vector.tensor_copy(out=pc[:, 0:1], in_=mv[:, 0:1])
            nc.vector.scalar_tensor_tensor(out=pc[:, 1:2], in0=mv[:, 0:1],
                                           scalar=mv[:, 0:1], in1=mv[:, 1:2],
                                           op0=OP.mult, op1=OP.add)  # mean^2 + var
            pcs.append(pc)
        # group-reduce across the 32 channels of each group (partition axis)
        pg = pp.tile([G, 2], F32, tag="ps")
        for ch in range(NCH):
            nc.tensor.matmul(pg, ind_t[ch], pcs[ch], start=(ch == 0), stop=(ch == NCH - 1))
        br8 = sm.tile([G, 2], F32, tag=f"br8_{b}")   # [:,0]=mean_g  [:,1]=rstd_g
        nc.vector.tensor_scalar_mul(out=br8, in0=pg, scalar1=1.0 / 32.0)
        m2g = sm.tile([G, 1], F32, tag="m2g")
        nc.vector.tensor_mul(m2g, br8[:, 0:1], br8[:, 0:1])
        nc.vector.tensor_sub(br8[:, 1:2], br8[:, 1:2], m2g)    # var_g
        nc.scalar.activation(out=br8[:, 1:2], in_=br8[:, 1:2], func=AF.Sqrt, bias=eps8, scale=1.0)
        nc.vector.reciprocal(out=br8[:, 1:2], in_=br8[:, 1:2])
        st[b]["br8"] = br8

    def emit_head_apply(b):
        # broadcast group stats back to channels, fold gn affine, normalize
        xt, br8 = st[b]["xt"], st[b]["br8"]
        ht = []
        for ch in range(NCH):
            pbc = pp.tile([P, 2], F32, tag="ps")
            nc.tensor.matmul(pbc, indT_t[ch], br8)
            s_ = sm.tile([P, 1], F32, tag=f"s{ch}")
            t_ = sm.tile([P, 1], F32, tag=f"t{ch}")
            nc.vector.tensor_mul(s_, pbc[:, 1:2], gnw_t[ch])   # s = rstd * w
            nc.vector.scalar_tensor_tensor(out=t_, in0=pbc[:, 0:1], scalar=s_,
                                           in1=gnb_t[ch], op0=OP.mult,
                                           op1=OP.subtract)    # t = mean*s - b
            h_ = pools["hp"].tile([P, N], SIG_DT, name=f"ht{ch}", tag=f"ht{ch}")
            nc.vector.tensor_scalar(
                out=h_, in0=xt[ch], scalar1=s_, scalar2=t_,
                op0=OP.mult, op1=OP.subtract)  # x*s - t
            ht.append(h_)
        st[b]["ht"] = ht

    def emit_qkv(b, v_on_dve=False):
        ht = st[b]["ht"]
        # -- q, k projections: [C, N] = W^T.T @ h (+ bias during PSUM move) --
        # i-half-major so attention on i-half 0 starts after only 4 moves
        qt = [pools["qk"].tile([P, N], SIG_DT, name=f"qt{och}", tag=f"qt{och}")
              for och in range(NCH)]
        kt = [pools["qk"].tile([P, N], SIG_DT, name=f"kt{och}", tag=f"kt{och}")
              for och in range(NCH)]
        for ih in range(NIH):
            for wname, dst in (("q", qt), ("k", kt)):
                for och in range(NCH):
                    pq = pp.tile([P, IH], F32, tag="ps")
                    for cch in range(NCH):
                        nc.tensor.matmul(
                            pq,
                            wt[wname][cch][:, och * P:(och + 1) * P],
                            ht[cch][:, ih * IH:(ih + 1) * IH],
                            start=(cch == 0), stop=(cch == NCH - 1))
                    if wname == "k":
                        nc.scalar.add(out=dst[och][:, ih * IH:(ih + 1) * IH],
                                      in_=pq, add=bt[wname][och])
                    else:
                        nc.vector.tensor_scalar_add(
                            out=dst[och][:, ih * IH:(ih + 1) * IH], in0=pq,
                            scalar1=bt[wname][och])

        # -- v, produced transposed: vT[n, o] = h[:, n].T @ wvT  (bv folded
        # into bp on the host: softmax weights sum to 1, so the +bv term
        # passes through attention unchanged and commutes with wp) --
        vt = []
        for j in range(NJ):
            pv = pp.tile([P, C], F32, tag="ps")
            for cch in range(NCH):
                nc.tensor.matmul(pv, ht[cch][:, j * P:(j + 1) * P], wt["v"][cch],
                                 start=(cch == 0), stop=(cch == NCH - 1))
            v_ = pools["vtp"].tile([P, C], VAL_DT, name=f"vt{j}", tag=f"vt{j}")
            # PSUM evacuations balanced between ACT and DVE; all-DVE for
            # batch 0 so the in-order ACT stream reaches the first Exp
            # without 8 copies queued ahead of it
            if v_on_dve or (j & 1):
                nc.vector.tensor_copy(out=v_, in_=pv)
            else:
                nc.scalar.copy(out=v_, in_=pv)
            vt.append(v_)
        st[b].update(qt=qt, kt=kt, vt=vt)

    def emit_attn_scores(b, ih):
        qt, kt, vt = (st[b][k] for k in ("qt", "kt", "vt"))
        if ih == 0:
            st[b]["fin"] = [pools["outp"].tile([P, N], F32, name=f"fin{och}",
                                               tag=f"fin{och}") for och in range(NCH)]
        isl = slice(ih * IH, (ih + 1) * IH)
        po = [pp.tile([P, IH], F32, name=f"po{_}", tag="ps") for _ in range(NCH)]
        ets = [None] * NJ
        # rowsum accumulators: two bf16 add-chains (even/odd j) + combine
        chain = {0: [], 1: []}

        def s_stage(j):
            ps = pp.tile([P, IH], F32, tag="ps")
            for cch in range(NCH):
                nc.tensor.matmul(ps,
                                 kt[cch][:, j * P:(j + 1) * P],
                                 qt[cch][:, isl],
                                 start=(cch == 0), stop=(cch == NCH - 1))
            et = pools["etp"].tile([P, IH], VAL_DT, name=f"et{j}", tag=f"et{j}")
            nc.scalar.activation(out=et, in_=ps, func=AF.Exp)
            ets[j] = et

        def es_add(j):
            # after et[j] lands, extend that parity's chain by one add
            par = j & 1
            if j < 2:
                chain[par].append(ets[j])
                return
            acc = pools["esp"].tile([P, IH], VAL_DT, tag=f"es{par}_{j // 2}")
            es_eng.tensor_add(acc, chain[par][-1], ets[j])
            chain[par].append(acc)

        def acc_stage(j):
            et = ets[j]
            if ESUM == 'pe':
                nc.tensor.matmul(prs, ones128, et, start=(j == 0), stop=(j == NJ - 1))
            for och in range(NCH):
                nc.tensor.matmul(po[och], vt[j][:, och * P:(och + 1) * P], et,
                                 start=(j == 0), stop=(j == NJ - 1))

        if ESUM == 'pe':
            prs = pp.tile([P, IH], F32, name="prs", tag="ps")
            s_stage(0)
            for j in range(1, NJ):
                s_stage(j)
                acc_stage(j - 1)
            acc_stage(NJ - 1)
        else:
            s_stage(0)
            for j in range(1, NJ):
                s_stage(j)
                es_add(j - 1)
                acc_stage(j - 1)
            es_add(NJ - 1)
            acc_stage(NJ - 1)
            esum = pools["esp"].tile([P, IH], VAL_DT, tag="esum")
            es_eng.tensor_add(esum, chain[0][-1], chain[1][-1])
            prs = pp.tile([P, IH], F32, name="prs", tag="ps")
            nc.tensor.matmul(prs, ones128, esum, start=True, stop=True)
        st[b][f"acc{ih}"] = (prs, po)

    def emit_attn_norm(b, ih, csl=None, cn=''):
        prs, po = st[b][f"acc{ih}"]
        csl = csl if csl is not None else slice(0, IH)
        w = csl.stop - csl.start
        rb = sm.tile([P, w], F32, tag="rb" + cn)
        rscratch = sm.tile([P, w], F32, tag="rscratch" + cn)
        nc.vector.reciprocal_approx_accurate(out=rb, in_=prs[:, csl], scratch=rscratch)
        ou = []
        for cch in range(NCH):
            o_ = pools["outp"].tile([P, w], VAL_DT, name=f"ou{cch}", tag=f"ou{cch}{cn}")
            nc.vector.tensor_mul(o_, po[cch][:, csl], rb)           # normalize
            ou.append(o_)
        st[b][f"ou{ih}{cn}"] = ou

    def emit_attn_out(b, ih, csl=None, cn=''):
        xt, fin = st[b]["xt"], st[b]["fin"]
        ou = st[b][f"ou{ih}{cn}"]
        csl = csl if csl is not None else slice(0, IH)
        isl = slice(ih * IH + csl.start, ih * IH + csl.stop)
        for och in range(NCH):
            pz = pp.tile([P, csl.stop - csl.start], F32, tag="ps")
            for cch in range(NCH):
                nc.tensor.matmul(pz,
                                 wt["p"][cch][:, och * P:(och + 1) * P],
                                 ou[cch],
                                 start=(cch == 0), stop=(cch == NCH - 1))
            # y = (wp@ou + bp') + x   in one fused DVE pass
            nc.vector.scalar_tensor_tensor(
                out=fin[och][:, isl], in0=pz, scalar=bt["p"][och],
                in1=xt[och][:, isl], op0=OP.add, op1=OP.add)
            nc.sync.dma_start(out=y_d[b, och * P:(och + 1) * P, isl],
                              in_=fin[och][:, isl])

    # Schedule: ALL four batches' GroupNorm stats run up front (their
    # ScalarE Sqrts land before the first attention Exp -> no ACT table
    # thrash), while the normalize ("apply") stage of batch b+2 and the
    # qkv of batch b+1 are pipelined into batch b's attention stream so
    # DVE work hides under the attention matmuls and the PE never idles
    # long enough for the HAM clock gate to re-throttle.
    emit_head_stats(0)
    emit_head_apply(0)
    emit_head_stats(1)
    emit_head_stats(2)
    emit_head_stats(3)
    emit_qkv(0, v_on_dve=True)
    emit_head_apply(1)
    for b in range(BPC):
        emit_attn_scores(b, 0)
        emit_attn_norm(b, 0)
        emit_attn_scores(b, 1)
        if b + 2 < BPC:
            emit_head_apply(b + 2)
        emit_attn_out(b, 0)
        if b + 1 < BPC:
            emit_attn_norm(b, 1)
            emit_qkv(b + 1)
            emit_attn_out(b, 1)
        else:
            # closing chain: chunk columns so norm/proj/residual/DMA pipeline
            cw = IH // TAIL_CHUNKS
            for c in range(TAIL_CHUNKS):
                csl = slice(c * cw, (c + 1) * cw)
                emit_attn_norm(b, 1, csl, cn=f"c{c}")
                emit_attn_out(b, 1, csl, cn=f"c{c}")
        del st[b]

    for k in reversed(list(ctxpools)):
        ctxpools[k].__exit__(None, None, None)


def build_bass():
    nc = bacc.Bacc("TRN2", target_bir_lowering=False, debug=False)
    x_d = nc.dram_tensor("x", [BPC, C, N], F32, kind="ExternalInput")
    wd = {name: nc.dram_tensor(f"w{name}T", [C, C], VAL_DT if name == "p" else SIG_DT,
                               kind="ExternalInput")
          for name in ("q", "k", "v", "p")}
    spack_d = nc.dram_tensor("spack", [P, 26], F32, kind="ExternalInput")
    indT_d = nc.dram_tensor("indT", [G, C], F32, kind="ExternalInput")
    ones_d = nc.dram_tensor("ones", [P, P], VAL_DT, kind="ExternalInput")
    y_d = nc.dram_tensor("y", [BPC, C, N], F32, kind="ExternalOutput")

    with tile.TileContext(nc) as tc:
        build_kernel_body(nc, tc, x_d, y_d, wd, spack_d, indT_d, ones_d)
    nc.compile()
    return nc


def host_inputs(inputs):
    """Per-core replicated constants from the full input dict."""
    import ml_dtypes
    np_sig = np.float32 if SIG_DT != BF16 else ml_dtypes.bfloat16
    np_val = np.float32 if VAL_DT != BF16 else ml_dtypes.bfloat16
    f = lambda a: np.ascontiguousarray(np.asarray(a), dtype=np.float32)
    scale = np.float32(C ** -0.5)
    ind = np.zeros((C, G), dtype=np.float32)
    for c in range(C):
        ind[c, c // GS] = 1.0
    bq = f(inputs["bq"]) * scale
    bk = f(inputs["bk"])
    # bv commutes through the softmax (weights sum to 1): fold wp@bv into bp
    bp = f(inputs["bp"]) + f(inputs["wp"]) @ f(inputs["bv"])
    gnw = f(inputs["gn_w"])
    gnb = f(inputs["gn_b"])
    spack = np.zeros((P, 26), dtype=np.float32)
    for ch in range(NCH):
        sl = slice(ch * P, (ch + 1) * P)
        spack[:, 0 + ch] = bq[sl]
        spack[:, 2 + ch] = bk[sl]
        spack[:, 4 + ch] = bp[sl]
        spack[:, 6 + ch] = gnw[sl]
        spack[:, 8 + ch] = gnb[sl]
        spack[:, 10 + 8 * ch:18 + 8 * ch] = ind[sl, :]
    consts = {
        "wqT": f(np.asarray(inputs["wq"], dtype=np.float32).T * scale).astype(np_sig),
        "wkT": f(np.asarray(inputs["wk"], dtype=np.float32).T).astype(np_sig),
        "wvT": f(np.asarray(inputs["wv"], dtype=np.float32).T).astype(np_sig),
        "wpT": f(np.asarray(inputs["wp"], dtype=np.float32).T).astype(np_val),
        "spack": spack,
        "indT": np.ascontiguousarray(ind.T),
        "ones": np.ones((P, P), dtype=np_val),
    }
    return consts


_NC_CACHE = []


def _get_nc():
    if not _NC_CACHE:
        _NC_CACHE.append(build_bass())
    return _NC_CACHE[0]


def kernel(trace=False, trace_cores=None, **inputs):
    nc = _get_nc()
    consts = host_inputs(inputs)
    x = np.ascontiguousarray(np.asarray(inputs["x"], dtype=np.float32)).reshape(B, C, N)
    in_maps = []
    for core in range(NCORES):
        m = dict(consts)
        m["x"] = np.ascontiguousarray(x[core * BPC:(core + 1) * BPC])
        in_maps.append(m)
    res = run_bass_kernel_spmd(nc, in_maps, core_ids=list(range(NCORES)),
                               trace=trace, trace_cores=trace_cores)
    y = np.concatenate([r["y"] for r in res.results], axis=0)
    out = y.reshape(B, C, HH, WW).astype(np.float32)
    if trace:
        return out, res
    return out
